# revision 1
# baseline (speedup 1.0000x reference)
"""DharmaAttention TRN2 kernel.

Full-input contract: kernel(**inputs) takes the unsharded inputs and returns
the full [2, 2048, 2048] output.

Sharding (8 cores): 2-way data-parallel over batch x 4-way tensor-parallel
over head groups (4 heads of head_dim 128 per core). Wq/Wk/Wv are split
column-wise (output channels) per head group, Wo row-wise; each core produces
a partial output projection for its batch element and the host sums the 4
partials per batch.

Per-core layouts (host-side prep, all fp32 bytes):
  xT   [2048, 2048]  hidden_states[b].T              (contraction dim on partitions)
  wqT  [2048, 512]   Wq[rows of group].T             (same for wkT, wvT)
  woc  [512, 2048]   Wo[:, cols of group].T
  cosT [128, 2048]   rope cos table, [d, s]
  sinN [128, 2048]   rows 0:64 = -sin, rows 64:128 = +sin, [d, s]
  maskd [128, 4, 512] binary causal masks for the 4 diagonal block offsets
Output:
  yT   [2048, 2048]  partial (Wo row-shard) output, transposed [o, s]

All matmuls run as float32r (full PE rate). Softmax skips the max
subtraction: scores are O(+-6), exp is safe in fp32, and softmax is
shift-invariant so the result matches the reference.
"""

import math
import sys

sys.path.insert(0, "/opt/trn_rl_repo")

import numpy as np

B = 2
S = 2048
H = 2048
NH = 16
HD = 128
THETA = 10000.0
G = 4  # heads per core (tensor-parallel group size NH / 4)
GC = G * HD  # channels per core = 512
NHT = H // 128  # 16 contraction tiles
SC = 512  # phase-0/1 seq chunk
NSC = S // SC  # 4
QC = 512  # attention q chunk
NQC = S // QC  # 4
NKB = S // 128  # 16 k blocks
INV_SQRT_HD = 1.0 / math.sqrt(HD)

_prog_cache = {}

# test-harness hooks (the grading path leaves these at defaults)
TRACE = False
LAST_RESULTS = None


def _split_multi_waits(nc):
    """The walrus build here accepts at most ONE sync wait per instruction
    ('Too many sync wait commands'). Hoist extra on_wait entries into no-op
    instructions inserted just before, on the same engine."""
    import concourse.mybir as mybir

    for f in nc.m.functions:
        for b in f.blocks:
            out = []
            changed = False
            for inst in b.instructions:
                si = getattr(inst, "sync_info", None)
                waits = list(si.on_wait) if si is not None and si.on_wait else []
                if len(waits) > 1:
                    for k, w in enumerate(waits[:-1]):
                        nop = mybir.InstNoOp(
                            name=f"{inst.name}-w{k}",
                            sync_info=mybir.SyncInfo(on_wait=[w], on_update=[]),
                        )
                        nop.engine = inst.engine
                        out.append(nop)
                    inst.sync_info = mybir.SyncInfo(
                        on_wait=[waits[-1]], on_update=list(si.on_update or [])
                    )
                    changed = True
                out.append(inst)
            if changed:
                b.instructions = out


def _build_nc():
    import concourse.bass as bass
    import concourse.mybir as mybir
    import concourse.tile as tile

    F32 = mybir.dt.float32
    F32R = mybir.dt.float32r
    MULT = mybir.AluOpType.mult
    ADD = mybir.AluOpType.add
    DIV = mybir.AluOpType.divide
    EXP = mybir.ActivationFunctionType.Exp

    nc = bass.Bass("TRN2", target_bir_lowering=False, debug=False)

    xT = nc.dram_tensor("xT", [H, S], F32R, kind="ExternalInput").ap()
    wqT = nc.dram_tensor("wqT", [H, GC], F32R, kind="ExternalInput").ap()
    wkT = nc.dram_tensor("wkT", [H, GC], F32R, kind="ExternalInput").ap()
    wvT = nc.dram_tensor("wvT", [H, GC], F32R, kind="ExternalInput").ap()
    woc = nc.dram_tensor("woc", [GC, H], F32R, kind="ExternalInput").ap()
    cosT_d = nc.dram_tensor("cosT", [HD, S], F32, kind="ExternalInput").ap()
    sinN_d = nc.dram_tensor("sinN", [HD, S], F32, kind="ExternalInput").ap()
    maskd_d = nc.dram_tensor("maskd", [128, 4, QC], F32, kind="ExternalInput").ap()
    yT = nc.dram_tensor("yT", [H, S], F32, kind="ExternalOutput").ap()

    with tile.TileContext(nc) as tc:
        with (
            tc.tile_pool(name="consts", bufs=1) as consts,
            tc.tile_pool(name="dram", bufs=1, space="DRAM") as dram,
        ):
            cosT = consts.tile([HD, S], F32)
            sinN = consts.tile([HD, S], F32)
            ones_f = consts.tile([128, 128], F32)
            ones_mat = consts.tile([128, 128], F32R)
            nc.sync.dma_start(out=cosT, in_=cosT_d)
            nc.sync.dma_start(out=sinN, in_=sinN_d)
            nc.vector.memset(ones_f, 1.0)
            nc.vector.tensor_copy(ones_mat, ones_f)

            qT_d = dram.tile([G, 128, S], F32R)
            kT_d = dram.tile([G, 128, S], F32R)
            v_d = dram.tile([NKB, 128, GC], F32R)

            # ---------------- Phase 0: V projection (first x pass) -----------
            with (
                tc.tile_pool(name="wvpool", bufs=1) as wvpool,
                tc.tile_pool(name="xvpool", bufs=2) as xvpool,
                tc.tile_pool(name="vstage", bufs=3) as vstage,
                tc.tile_pool(name="ps0", bufs=1, space="PSUM") as ps0,
            ):
                wv_sb = wvpool.tile([128, NHT, GC], F32R)
                nc.sync.dma_start(out=wv_sb, in_=wvT.rearrange("(t p) o -> p t o", p=128))
                for sc in range(NSC):
                    ssl = slice(sc * SC, (sc + 1) * SC)
                    xv_sb = xvpool.tile([128, NHT, SC], F32R)
                    nc.sync.dma_start(
                        out=xv_sb, in_=xT[:, ssl].rearrange("(t p) s -> p t s", p=128)
                    )
                    for st2 in range(SC // 128):
                        st = sc * (SC // 128) + st2
                        pv = ps0.tile([128, GC], F32, tag="pv", bufs=4)
                        for ht in range(NHT):
                            nc.tensor.matmul(
                                pv,
                                xv_sb[:, ht, st2 * 128 : (st2 + 1) * 128],
                                wv_sb[:, ht, :],
                                start=(ht == 0),
                                stop=(ht == NHT - 1),
                            )
                        vst = vstage.tile([128, GC], F32R)
                        nc.scalar.copy(vst, pv)
                        nc.sync.dma_start(out=v_d[st], in_=vst)

            # ---------------- Phase 1: Q/K projections + RoPE (second x pass)
            with (
                tc.tile_pool(name="wpool", bufs=1) as wpool,
                tc.tile_pool(name="xpool", bufs=2) as xpool,
                tc.tile_pool(name="rpool", bufs=3) as rpool,
                tc.tile_pool(name="dpool", bufs=3) as dpool,
                tc.tile_pool(name="ps1", bufs=1, space="PSUM") as ps1,
            ):
                wq_sb = wpool.tile([128, NHT, GC], F32R)
                wk_sb = wpool.tile([128, NHT, GC], F32R)
                nc.sync.dma_start(out=wq_sb, in_=wqT.rearrange("(t p) o -> p t o", p=128))
                nc.sync.dma_start(out=wk_sb, in_=wkT.rearrange("(t p) o -> p t o", p=128))

                for sc in range(NSC):
                    ssl = slice(sc * SC, (sc + 1) * SC)
                    x_sb = xpool.tile([128, NHT, SC], F32R)
                    nc.sync.dma_start(
                        out=x_sb, in_=xT[:, ssl].rearrange("(t p) s -> p t s", p=128)
                    )
                    for h in range(G):
                        for w_sb, dst_d in ((wq_sb, qT_d), (wk_sb, kT_d)):
                            pqk = ps1.tile([128, SC], F32, tag="pqk", bufs=6)
                            for ht in range(NHT):
                                nc.tensor.matmul(
                                    pqk,
                                    w_sb[:, ht, h * 128 : (h + 1) * 128],
                                    x_sb[:, ht, :],
                                    start=(ht == 0),
                                    stop=(ht == NHT - 1),
                                )
                            # RoPE: dst = pqk * cos + rot_half(pqk) * sin
                            tmp = rpool.tile([128, SC], F32)
                            nc.vector.tensor_tensor(
                                out=tmp[0:64, :], in0=pqk[64:128, :],
                                in1=sinN[0:64, ssl], op=MULT,
                            )
                            nc.vector.tensor_tensor(
                                out=tmp[64:128, :], in0=pqk[0:64, :],
                                in1=sinN[64:128, ssl], op=MULT,
                            )
                            cpart = rpool.tile([128, SC], F32, tag="cpart")
                            nc.vector.tensor_tensor(
                                out=cpart, in0=pqk, in1=cosT[:, ssl], op=MULT
                            )
                            dst = dpool.tile([128, SC], F32R)
                            nc.vector.tensor_tensor(out=dst, in0=cpart, in1=tmp, op=ADD)
                            nc.sync.dma_start(out=dst_d[h, :, ssl], in_=dst)

            # ---------------- Phase 2: attention; Phase 3: out projection ----
            with (
                tc.tile_pool(name="qkpool", bufs=2) as qkpool,
                tc.tile_pool(name="vhpool", bufs=2) as vhpool,
                tc.tile_pool(name="outpool", bufs=4) as outpool,
                tc.tile_pool(name="prpool", bufs=4) as prpool,
                tc.tile_pool(name="bcpool", bufs=2) as bcpool,
                tc.tile_pool(name="maskpool", bufs=1) as maskpool,
                tc.tile_pool(name="wopool", bufs=1) as wopool,
                tc.tile_pool(name="ystage", bufs=2) as ystage,
            ):
                maskd = maskpool.tile([128, 4, QC], F32)
                nc.sync.dma_start(out=maskd, in_=maskd_d)
                woc_sb = wopool.tile([128, G, H], F32R)
                nc.sync.dma_start(
                    out=woc_sb, in_=woc.rearrange("(c p) o -> p c o", p=128)
                )

                out_h = []
                with tc.tile_pool(name="ps2", bufs=1, space="PSUM") as ps2:
                    for h in range(G):
                        qh = qkpool.tile([128, S], F32R, tag="qh")
                        kh = qkpool.tile([128, S], F32R, tag="kh")
                        vh = vhpool.tile([128, NKB, 128], F32R)
                        # chunked loads so the first q-chunk starts early
                        for qc in range(NQC):
                            qsl = slice(qc * QC, (qc + 1) * QC)
                            nc.sync.dma_start(out=qh[:, qsl], in_=qT_d[h][:, qsl])
                            nc.sync.dma_start(out=kh[:, qsl], in_=kT_d[h][:, qsl])
                            nc.sync.dma_start(
                                out=vh[:, 4 * qc : 4 * qc + 4, :],
                                in_=v_d[
                                    4 * qc : 4 * qc + 4, :, h * 128 : (h + 1) * 128
                                ].transpose([1, 0, 2]),
                            )
                        outh = outpool.tile([128, S], F32R, tag="outh")
                        out_h.append(outh)
                        for qc in range(NQC):
                            qsl = slice(qc * QC, (qc + 1) * QC)
                            nk = 4 * qc + 4
                            po = ps2.tile([128, QC], F32, tag="po", bufs=3)
                            # sums broadcast to all 128 rows via all-ones lhsT
                            pbs = ps2.tile([128, QC], F32, tag="pbs", bufs=3)
                            for ki in range(nk):
                                psc = ps2.tile([128, QC], F32, tag="psc", bufs=2)
                                nc.tensor.matmul(
                                    psc,
                                    kh[:, ki * 128 : (ki + 1) * 128],
                                    qh[:, qsl],
                                    start=True,
                                    stop=True,
                                )
                                pr = prpool.tile([128, QC], F32R, tag="pr")
                                m = ki - 4 * qc
                                if m >= 0:
                                    prf = prpool.tile([128, QC], F32, tag="prf")
                                    nc.scalar.activation(
                                        prf, psc, EXP, scale=INV_SQRT_HD
                                    )
                                    nc.vector.tensor_tensor(
                                        out=pr, in0=prf, in1=maskd[:, m, :], op=MULT
                                    )
                                else:
                                    nc.scalar.activation(
                                        pr, psc, EXP, scale=INV_SQRT_HD
                                    )
                                nc.tensor.matmul(
                                    po, vh[:, ki, :], pr,
                                    start=(ki == 0), stop=(ki == nk - 1),
                                )
                                nc.tensor.matmul(
                                    pbs, ones_mat, pr,
                                    start=(ki == 0), stop=(ki == nk - 1),
                                )
                            bc = bcpool.tile([128, QC], F32)
                            nc.vector.reciprocal(out=bc, in_=pbs)
                            nc.vector.tensor_tensor(
                                out=outh[:, qsl], in0=po, in1=bc, op=MULT
                            )

                with tc.tile_pool(name="ps3", bufs=1, space="PSUM") as ps3:
                    for ot in range(NHT):
                        ysf = ystage.tile([128, S], F32)
                        for sch in range(NQC):
                            ssl = slice(sch * QC, (sch + 1) * QC)
                            py = ps3.tile([128, QC], F32, tag="py", bufs=4)
                            for h in range(G):
                                nc.tensor.matmul(
                                    py,
                                    woc_sb[:, h, ot * 128 : (ot + 1) * 128],
                                    out_h[h][:, ssl],
                                    start=(h == 0),
                                    stop=(h == G - 1),
                                )
                            nc.scalar.copy(ysf[:, ssl], py)
                        nc.scalar.dma_start(
                            out=yT[ot * 128 : (ot + 1) * 128, :], in_=ysf
                        )
    _split_multi_waits(nc)
    return nc


def _host_tables():
    inv_freq = 1.0 / (THETA ** (np.arange(0, HD, 2, dtype=np.float32) / HD))
    t = np.arange(S, dtype=np.float32)
    freqs = np.einsum("i,j->ij", t, inv_freq)  # [S, 64]
    cos_h = np.cos(freqs).astype(np.float32)  # [S, 64]
    sin_h = np.sin(freqs).astype(np.float32)
    cosT = np.empty((HD, S), np.float32)
    cosT[0:64] = cos_h.T
    cosT[64:128] = cos_h.T
    sinN = np.empty((HD, S), np.float32)
    sinN[0:64] = -sin_h.T
    sinN[64:128] = sin_h.T
    p = np.arange(128)[:, None]
    s = np.arange(QC)[None, :]
    maskd = np.empty((128, 4, QC), np.float32)
    for m in range(4):
        maskd[:, m, :] = (s >= 128 * m + p).astype(np.float32)
    return cosT, sinN, maskd


def kernel(hidden_states, Wq, Wk, Wv, Wo):
    from concourse import bass_utils

    hidden_states = np.asarray(hidden_states, dtype=np.float32)
    Wq = np.asarray(Wq, dtype=np.float32)
    Wk = np.asarray(Wk, dtype=np.float32)
    Wv = np.asarray(Wv, dtype=np.float32)
    Wo = np.asarray(Wo, dtype=np.float32)

    if "nc" not in _prog_cache:
        _prog_cache["nc"] = _build_nc()
    nc = _prog_cache["nc"]

    cosT, sinN, maskd = _host_tables()
    in_maps = []
    for c in range(8):
        b, g = divmod(c, 4)
        rows = slice(g * GC, (g + 1) * GC)
        in_maps.append(
            {
                "xT": np.ascontiguousarray(hidden_states[b].T),
                "wqT": np.ascontiguousarray(Wq[rows, :].T),
                "wkT": np.ascontiguousarray(Wk[rows, :].T),
                "wvT": np.ascontiguousarray(Wv[rows, :].T),
                "woc": np.ascontiguousarray(Wo[:, rows].T),
                "cosT": cosT,
                "sinN": sinN,
                "maskd": maskd,
            }
        )

    res = bass_utils.run_bass_kernel_spmd(
        nc, in_maps, core_ids=list(range(8)), trace=TRACE
    )
    global LAST_RESULTS
    LAST_RESULTS = res

    out = np.zeros((B, S, H), np.float32)
    for c in range(8):
        b = c // 4
        out[b] += res.results[c]["yT"].T
    return out



# revision 5
# speedup vs baseline: 1.3060x; 1.3060x over previous
"""DharmaAttention TRN2 kernel — fused single-pass bf16 pipeline.

Full-input contract: kernel(**inputs) takes the unsharded inputs and returns
the full [2, 2048, 2048] fp32 output.

Sharding (8 cores): 2-way data-parallel over batch x 4-way tensor-parallel
over head groups (4 heads of head_dim 128 per core). Wq/Wk/Wv split
column-wise per head group, Wo row-wise; host sums the 4 partial output
projections per batch element.

v2 design (vs the phase-split fp32r baseline):
  - everything bf16 on the wire and in SBUF (halves DMA + SBUF, enables FWL
    weight loads and 2x DVE modes); PSUM accumulation stays fp32.
  - ONE fused pass per 512-token seq chunk: Q/K proj + RoPE -> V proj ->
    causal attention for that q chunk (k/v of chunks 0..sc stay SBUF
    resident, no DRAM round trip) -> output projection -> DMA out.
  - causal mask applied by an extra accumulate-matmul (-340*I @ B_m) into
    the score PSUM group instead of a DVE multiply; exp then yields ~0.
  - diagonal score blocks only compute the live q range (512-128m cols).
  - softmax denominator: ones-matmul accumulated in PSUM (as baseline), but
    1/x via reciprocal_approx_fast (~5x faster than exact reciprocal).

Per-core DRAM layouts (all bf16):
  xT   [2048, 2048]  hidden_states[b].T          (contraction on partitions)
  wqT  [2048, 512]   Wq[rows of group].T         (same wkT, wvT)
  woc  [512, 2048]   Wo[:, cols of group].T
  cosb [128, 2048]   rope cos table [d, s]
  sinb [128, 2048]   rows 0:64 = -sin, rows 64:128 = +sin
  bneg [128, 4, 512] causal 0/1 tables per diagonal offset m
  iden [128, 128]    -340 * I   (mask add via PE)
  ones [128, 128]    all-ones   (softmax denominator via PE)
Output:
  yT   [2048, 2048]  partial (Wo row-shard) output, [o, s], bf16
"""

import math
import sys

sys.path.insert(0, "/opt/trn_rl_repo")

import numpy as np

B = 2
S = 2048
H = 2048
NH = 16
HD = 128
THETA = 10000.0
G = 4  # heads per core
GC = G * HD  # 512 channels per core
NHT = H // 128  # 16 contraction tiles
SC = 512  # seq chunk
NSC = S // SC  # 4
INV_SQRT_HD = 1.0 / math.sqrt(HD)
MASKVAL = -340.0  # * INV_SQRT_HD ~= -30 after the exp scale

_prog_cache = {}

# test-harness hooks (the grading path leaves these at defaults)
TRACE = False
LAST_RESULTS = None


def _split_multi_waits(nc):
    """The walrus build here accepts at most ONE sync wait per instruction
    ('Too many sync wait commands'). Hoist extra on_wait entries into no-op
    instructions inserted just before, on the same engine."""
    import concourse.mybir as mybir

    for f in nc.m.functions:
        for b in f.blocks:
            out = []
            changed = False
            for inst in b.instructions:
                si = getattr(inst, "sync_info", None)
                waits = list(si.on_wait) if si is not None and si.on_wait else []
                if len(waits) > 1:
                    for k, w in enumerate(waits[:-1]):
                        nop = mybir.InstNoOp(
                            name=f"{inst.name}-w{k}",
                            sync_info=mybir.SyncInfo(on_wait=[w], on_update=[]),
                        )
                        nop.engine = inst.engine
                        out.append(nop)
                    inst.sync_info = mybir.SyncInfo(
                        on_wait=[waits[-1]], on_update=list(si.on_update or [])
                    )
                    changed = True
                out.append(inst)
            if changed:
                b.instructions = out


def _build_nc():
    import concourse.bass as bass
    import concourse.mybir as mybir
    import concourse.tile as tile

    F32 = mybir.dt.float32
    BF = mybir.dt.bfloat16
    MULT = mybir.AluOpType.mult
    ADD = mybir.AluOpType.add
    EXP = mybir.ActivationFunctionType.Exp
    LN = mybir.ActivationFunctionType.Ln

    nc = bass.Bass("TRN2", target_bir_lowering=False, debug=False)

    xT = nc.dram_tensor("xT", [H, S], BF, kind="ExternalInput").ap()
    wqT = nc.dram_tensor("wqT", [H, GC], BF, kind="ExternalInput").ap()
    wkT = nc.dram_tensor("wkT", [H, GC], BF, kind="ExternalInput").ap()
    wvT = nc.dram_tensor("wvT", [H, GC], BF, kind="ExternalInput").ap()
    woc = nc.dram_tensor("woc", [GC, H], BF, kind="ExternalInput").ap()
    cosb_d = nc.dram_tensor("cosb", [HD, S], BF, kind="ExternalInput").ap()
    sinb_d = nc.dram_tensor("sinb", [HD, S], BF, kind="ExternalInput").ap()
    bneg_d = nc.dram_tensor("bneg", [HD, 4, SC], BF, kind="ExternalInput").ap()
    iden_d = nc.dram_tensor("iden", [128, 128], BF, kind="ExternalInput").ap()
    ones_d = nc.dram_tensor("ones", [128, 128], BF, kind="ExternalInput").ap()
    yT = nc.dram_tensor("yT", [H, S], BF, kind="ExternalOutput").ap()

    with tile.TileContext(nc) as tc:
        with (
            tc.tile_pool(name="wpool", bufs=1) as wpool,
            tc.tile_pool(name="consts", bufs=1) as consts,
            tc.tile_pool(name="kvpool", bufs=1) as kvpool,
            tc.tile_pool(name="xpool", bufs=2) as xpool,
            tc.tile_pool(name="qpool", bufs=2) as qpool,
            tc.tile_pool(name="rpool", bufs=1) as rpool,
            tc.tile_pool(name="prpool", bufs=1) as prpool,
            tc.tile_pool(name="opool", bufs=2) as opool,
            tc.tile_pool(name="ypool", bufs=1) as ypool,
            tc.tile_pool(name="ps", bufs=1, space="PSUM") as ps,
        ):
            # weights first: wq/wk are on the critical path of chunk 0
            wq_sb = wpool.tile([128, NHT, GC], BF, tag="wq")
            nc.sync.dma_start(out=wq_sb, in_=wqT.rearrange("(t p) o -> p t o", p=128))
            wk_sb = wpool.tile([128, NHT, GC], BF, tag="wk")
            nc.sync.dma_start(out=wk_sb, in_=wkT.rearrange("(t p) o -> p t o", p=128))
            wv_sb = wpool.tile([128, NHT, GC], BF, tag="wv")
            nc.sync.dma_start(out=wv_sb, in_=wvT.rearrange("(t p) o -> p t o", p=128))

            cos_sb = consts.tile([HD, S], BF, tag="cos")
            sin_sb = consts.tile([HD, S], BF, tag="sin")
            bneg_sb = consts.tile([HD, 4, SC], BF, tag="bneg")
            iden_sb = consts.tile([128, 128], BF, tag="iden")
            ones_sb = consts.tile([128, 128], BF, tag="ones")
            nc.sync.dma_start(out=cos_sb, in_=cosb_d)
            nc.sync.dma_start(out=sin_sb, in_=sinb_d)
            nc.sync.dma_start(out=bneg_sb, in_=bneg_d)
            nc.sync.dma_start(out=iden_sb, in_=iden_d)
            nc.sync.dma_start(out=ones_sb, in_=ones_d)

            woc_sb = wpool.tile([128, G, H], BF, tag="woc")
            nc.sync.dma_start(out=woc_sb, in_=woc.rearrange("(c p) o -> p c o", p=128))

            k_chunks = []
            v_chunks = []
            for sc in range(NSC):
                ssl = slice(sc * SC, (sc + 1) * SC)
                x_sb = xpool.tile([128, NHT, SC], BF, tag="x")
                nc.sync.dma_start(
                    out=x_sb, in_=xT[:, ssl].rearrange("(t p) s -> p t s", p=128)
                )

                q_sb = qpool.tile([HD, G, SC], BF, tag="q")
                k_c = kvpool.tile([HD, G, SC], BF, tag=f"k{sc}")
                v_c = kvpool.tile([128, 4, GC], BF, tag=f"v{sc}")
                k_chunks.append(k_c)
                v_chunks.append(v_c)

                # ---- Q/K projections + RoPE ----
                for h in range(G):
                    for w_sb, dst in ((wq_sb, q_sb[:, h, :]), (wk_sb, k_c[:, h, :])):
                        pqk = ps.tile([128, SC], F32, tag="proj", bufs=2)
                        for ht in range(NHT):
                            nc.tensor.matmul(
                                pqk,
                                w_sb[:, ht, h * 128 : (h + 1) * 128],
                                x_sb[:, ht, :],
                                start=(ht == 0),
                                stop=(ht == NHT - 1),
                            )
                        # rope TTs read the PSUM directly: SBUF-SBUF TTs may
                        # not cross partition bases (walrus NCC_IBIR297)
                        tmp = rpool.tile([128, SC], BF, tag="tmp", bufs=2)
                        nc.vector.tensor_tensor(
                            out=tmp[0:64, :], in0=pqk[64:128, :],
                            in1=sin_sb[0:64, ssl], op=MULT,
                        )
                        nc.vector.tensor_tensor(
                            out=tmp[64:128, :], in0=pqk[0:64, :],
                            in1=sin_sb[64:128, ssl], op=MULT,
                        )
                        cp = rpool.tile([128, SC], BF, tag="cp", bufs=2)
                        nc.vector.tensor_tensor(
                            out=cp, in0=pqk, in1=cos_sb[:, ssl], op=MULT
                        )
                        nc.vector.tensor_tensor(out=dst, in0=cp, in1=tmp, op=ADD)

                # ---- V projection ----
                for st2 in range(4):
                    pv = ps.tile([128, SC], F32, tag="proj", bufs=2)
                    for ht in range(NHT):
                        nc.tensor.matmul(
                            pv,
                            x_sb[:, ht, st2 * 128 : (st2 + 1) * 128],
                            wv_sb[:, ht, :],
                            start=(ht == 0),
                            stop=(ht == NHT - 1),
                        )
                    nc.scalar.copy(v_c[:, st2, :], pv)

                # ---- causal attention for q chunk sc ----
                nk = 4 * sc + 4
                outh = opool.tile([HD, G, SC], BF, tag="outh")
                for h in range(G):
                    po = ps.tile([128, SC], F32, tag="po", bufs=2)
                    pbs = ps.tile([128, SC], F32, tag="pbs", bufs=2)
                    for ki in range(nk):
                        kc, kb = divmod(ki, 4)
                        m = ki - 4 * sc
                        qlo = 128 * m if m >= 0 else 0
                        qs = slice(qlo, SC)
                        psc = ps.tile([128, SC], F32, tag="att", bufs=2)
                        nc.tensor.matmul(
                            psc[:, qs],
                            k_chunks[kc][:, h, kb * 128 : (kb + 1) * 128],
                            q_sb[:, h, qs],
                            start=True,
                            stop=(m < 0),
                        )
                        if m >= 0:
                            nc.tensor.matmul(
                                psc[:, qs],
                                iden_sb,
                                bneg_sb[:, m, qs],
                                start=False,
                                stop=True,
                            )
                        pr = prpool.tile([128, SC], BF, tag="pr", bufs=3)
                        nc.scalar.activation(
                            pr[:, qs], psc[:, qs], EXP, scale=INV_SQRT_HD
                        )
                        nc.tensor.matmul(
                            po[:, qs],
                            v_chunks[kc][:, kb, h * 128 : (h + 1) * 128],
                            pr[:, qs],
                            start=(ki == 0),
                            stop=(ki == nk - 1),
                        )
                        nc.tensor.matmul(
                            pbs[:, qs],
                            ones_sb,
                            pr[:, qs],
                            start=(ki == 0),
                            stop=(ki == nk - 1),
                        )
                    # 1/x as exp(-ln(x)) on ACT: the custom-DVE fast
                    # reciprocal doesn't lower in this walrus build, and the
                    # exact DVE reciprocal costs 3.3us per tile.
                    lnb = rpool.tile([128, SC], F32, tag="lnb", bufs=2)
                    nc.scalar.activation(lnb, pbs, LN)
                    bc = rpool.tile([128, SC], F32, tag="bc", bufs=2)
                    nc.scalar.activation(bc, lnb, EXP, scale=-1.0)
                    nc.vector.tensor_tensor(
                        out=outh[:, h, :], in0=po, in1=bc, op=MULT
                    )

                # ---- output projection for chunk sc ----
                for ot in range(NHT):
                    py = ps.tile([128, SC], F32, tag="att", bufs=2)
                    for h in range(G):
                        nc.tensor.matmul(
                            py,
                            woc_sb[:, h, ot * 128 : (ot + 1) * 128],
                            outh[:, h, :],
                            start=(h == 0),
                            stop=(h == G - 1),
                        )
                    ysf = ypool.tile([128, SC], BF, tag="ysf", bufs=3)
                    nc.vector.tensor_copy(ysf, py)
                    nc.sync.dma_start(
                        out=yT[ot * 128 : (ot + 1) * 128, ssl], in_=ysf
                    )

    _split_multi_waits(nc)
    return nc


def _host_tables():
    import ml_dtypes

    BFN = ml_dtypes.bfloat16
    inv_freq = 1.0 / (THETA ** (np.arange(0, HD, 2, dtype=np.float32) / HD))
    t = np.arange(S, dtype=np.float32)
    freqs = np.einsum("i,j->ij", t, inv_freq)  # [S, 64]
    cos_h = np.cos(freqs).astype(np.float32)  # [S, 64]
    sin_h = np.sin(freqs).astype(np.float32)
    cosb = np.empty((HD, S), np.float32)
    cosb[0:64] = cos_h.T
    cosb[64:128] = cos_h.T
    sinb = np.empty((HD, S), np.float32)
    sinb[0:64] = -sin_h.T
    sinb[64:128] = sin_h.T
    p = np.arange(128)[:, None]
    q = np.arange(SC)[None, :]
    bneg = np.empty((128, 4, SC), np.float32)
    for m in range(4):
        bneg[:, m, :] = (q < 128 * m + p).astype(np.float32)
    iden = np.eye(128, dtype=np.float32) * MASKVAL
    ones = np.ones((128, 128), np.float32)
    return {
        "cosb": cosb.astype(BFN),
        "sinb": sinb.astype(BFN),
        "bneg": bneg.astype(BFN),
        "iden": iden.astype(BFN),
        "ones": ones.astype(BFN),
    }


def _in_maps(hidden_states, Wq, Wk, Wv, Wo):
    import ml_dtypes

    BFN = ml_dtypes.bfloat16
    tables = _host_tables()
    maps = []
    for c in range(8):
        b, g = divmod(c, 4)
        rows = slice(g * GC, (g + 1) * GC)
        maps.append(
            {
                "xT": np.ascontiguousarray(hidden_states[b].T).astype(BFN),
                "wqT": np.ascontiguousarray(Wq[rows, :].T).astype(BFN),
                "wkT": np.ascontiguousarray(Wk[rows, :].T).astype(BFN),
                "wvT": np.ascontiguousarray(Wv[rows, :].T).astype(BFN),
                "woc": np.ascontiguousarray(Wo[:, rows].T).astype(BFN),
                **tables,
            }
        )
    return maps


def kernel(hidden_states, Wq, Wk, Wv, Wo):
    from concourse import bass_utils

    hidden_states = np.asarray(hidden_states, dtype=np.float32)
    Wq = np.asarray(Wq, dtype=np.float32)
    Wk = np.asarray(Wk, dtype=np.float32)
    Wv = np.asarray(Wv, dtype=np.float32)
    Wo = np.asarray(Wo, dtype=np.float32)

    if "nc" not in _prog_cache:
        _prog_cache["nc"] = _build_nc()
    nc = _prog_cache["nc"]

    in_maps = _in_maps(hidden_states, Wq, Wk, Wv, Wo)
    res = bass_utils.run_bass_kernel_spmd(
        nc, in_maps, core_ids=list(range(8)), trace=TRACE
    )
    global LAST_RESULTS
    LAST_RESULTS = res

    out = np.zeros((B, S, H), np.float32)
    for c in range(8):
        b = c // 4
        out[b] += res.results[c]["yT"].T.astype(np.float32)
    return out


# revision 7
# speedup vs baseline: 1.3455x; 1.0303x over previous
"""DharmaAttention TRN2 kernel — fused single-pass bf16 pipeline.

Full-input contract: kernel(**inputs) takes the unsharded inputs and returns
the full [2, 2048, 2048] fp32 output.

Sharding (8 cores): 2-way data-parallel over batch x 4-way tensor-parallel
over head groups (4 heads of head_dim 128 per core). Wq/Wk/Wv split
column-wise per head group, Wo row-wise; host sums the 4 partial output
projections per batch element.

v2 design (vs the phase-split fp32r baseline):
  - everything bf16 on the wire and in SBUF (halves DMA + SBUF, enables FWL
    weight loads and 2x DVE modes); PSUM accumulation stays fp32.
  - ONE fused pass per 512-token seq chunk: Q/K proj + RoPE -> V proj ->
    causal attention for that q chunk (k/v of chunks 0..sc stay SBUF
    resident, no DRAM round trip) -> output projection -> DMA out.
  - causal mask applied by an extra accumulate-matmul (-340*I @ B_m) into
    the score PSUM group instead of a DVE multiply; exp then yields ~0.
  - diagonal score blocks only compute the live q range (512-128m cols).
  - softmax denominator: ones-matmul accumulated in PSUM (as baseline), but
    1/x via reciprocal_approx_fast (~5x faster than exact reciprocal).

Per-core DRAM layouts (all bf16):
  xT   [2048, 2048]  hidden_states[b].T          (contraction on partitions)
  wqT  [2048, 512]   Wq[rows of group].T         (same wkT, wvT)
  woc  [512, 2048]   Wo[:, cols of group].T
  cosb [128, 2048]   rope cos table [d, s]
  sinb [128, 2048]   rows 0:64 = -sin, rows 64:128 = +sin
  bneg [128, 4, 512] causal 0/1 tables per diagonal offset m
  iden [128, 128]    -340 * I   (mask add via PE)
  ones [128, 128]    all-ones   (softmax denominator via PE)
Output:
  yT   [2048, 2048]  partial (Wo row-shard) output, [o, s], bf16
"""

import math
import sys

sys.path.insert(0, "/opt/trn_rl_repo")

import numpy as np

B = 2
S = 2048
H = 2048
NH = 16
HD = 128
THETA = 10000.0
G = 4  # heads per core
GC = G * HD  # 512 channels per core
NHT = H // 128  # 16 contraction tiles
SC = 512  # seq chunk
NSC = S // SC  # 4
INV_SQRT_HD = 1.0 / math.sqrt(HD)
MASKVAL = -340.0  # * INV_SQRT_HD ~= -30 after the exp scale

_prog_cache = {}

# test-harness hooks (the grading path leaves these at defaults)
TRACE = False
LAST_RESULTS = None


def _split_multi_waits(nc):
    """The walrus build here accepts at most ONE sync wait per instruction
    ('Too many sync wait commands'). Hoist extra on_wait entries into no-op
    instructions inserted just before, on the same engine."""
    import concourse.mybir as mybir

    for f in nc.m.functions:
        for b in f.blocks:
            out = []
            changed = False
            for inst in b.instructions:
                si = getattr(inst, "sync_info", None)
                waits = list(si.on_wait) if si is not None and si.on_wait else []
                if len(waits) > 1:
                    for k, w in enumerate(waits[:-1]):
                        nop = mybir.InstNoOp(
                            name=f"{inst.name}-w{k}",
                            sync_info=mybir.SyncInfo(on_wait=[w], on_update=[]),
                        )
                        nop.engine = inst.engine
                        out.append(nop)
                    inst.sync_info = mybir.SyncInfo(
                        on_wait=[waits[-1]], on_update=list(si.on_update or [])
                    )
                    changed = True
                out.append(inst)
            if changed:
                b.instructions = out


def _build_nc():
    import concourse.bass as bass
    import concourse.mybir as mybir
    import concourse.tile as tile

    F32 = mybir.dt.float32
    BF = mybir.dt.bfloat16
    MULT = mybir.AluOpType.mult
    ADD = mybir.AluOpType.add
    EXP = mybir.ActivationFunctionType.Exp
    LN = mybir.ActivationFunctionType.Ln

    nc = bass.Bass("TRN2", target_bir_lowering=False, debug=False)

    xT = nc.dram_tensor("xT", [H, S], BF, kind="ExternalInput").ap()
    wqT = nc.dram_tensor("wqT", [H, GC], BF, kind="ExternalInput").ap()
    wkT = nc.dram_tensor("wkT", [H, GC], BF, kind="ExternalInput").ap()
    wvT = nc.dram_tensor("wvT", [H, GC], BF, kind="ExternalInput").ap()
    woc = nc.dram_tensor("woc", [GC, H], BF, kind="ExternalInput").ap()
    cosb_d = nc.dram_tensor("cosb", [HD, S], BF, kind="ExternalInput").ap()
    sinb_d = nc.dram_tensor("sinb", [HD, S], BF, kind="ExternalInput").ap()
    bneg_d = nc.dram_tensor("bneg", [HD, 4, SC], BF, kind="ExternalInput").ap()
    iden_d = nc.dram_tensor("iden", [128, 128], BF, kind="ExternalInput").ap()
    ones_d = nc.dram_tensor("ones", [128, 128], BF, kind="ExternalInput").ap()
    yT = nc.dram_tensor("yT", [H, S], BF, kind="ExternalOutput").ap()

    with tile.TileContext(nc) as tc:
        with (
            tc.tile_pool(name="wpool", bufs=1) as wpool,
            tc.tile_pool(name="consts", bufs=1) as consts,
            tc.tile_pool(name="kvpool", bufs=1) as kvpool,
            tc.tile_pool(name="xpool", bufs=2) as xpool,
            tc.tile_pool(name="qpool", bufs=2) as qpool,
            tc.tile_pool(name="rpool", bufs=1) as rpool,
            tc.tile_pool(name="prpool", bufs=1) as prpool,
            tc.tile_pool(name="opool", bufs=2) as opool,
            tc.tile_pool(name="ypool", bufs=1) as ypool,
            tc.tile_pool(name="ps", bufs=1, space="PSUM") as ps,
        ):
            # DMA issue order is the chunk-0 critical path: the Q projection
            # of chunk 0 needs only wq + x(0), so those go first; everything
            # else streams in behind them while the PE is already busy.
            wq_sb = wpool.tile([128, NHT, GC], BF, tag="wq")
            nc.sync.dma_start(out=wq_sb, in_=wqT.rearrange("(t p) o -> p t o", p=128))
            x0_sb = xpool.tile([128, NHT, SC], BF, tag="x")
            nc.sync.dma_start(
                out=x0_sb, in_=xT[:, 0:SC].rearrange("(t p) s -> p t s", p=128)
            )
            wk_sb = wpool.tile([128, NHT, GC], BF, tag="wk")
            nc.sync.dma_start(out=wk_sb, in_=wkT.rearrange("(t p) o -> p t o", p=128))
            wv_sb = wpool.tile([128, NHT, GC], BF, tag="wv")
            nc.sync.dma_start(out=wv_sb, in_=wvT.rearrange("(t p) o -> p t o", p=128))

            cos_sb = consts.tile([HD, S], BF, tag="cos")
            sin_sb = consts.tile([HD, S], BF, tag="sin")
            bneg_sb = consts.tile([HD, 4, SC], BF, tag="bneg")
            iden_sb = consts.tile([128, 128], BF, tag="iden")
            ones_sb = consts.tile([128, 128], BF, tag="ones")
            nc.sync.dma_start(out=cos_sb, in_=cosb_d)
            nc.sync.dma_start(out=sin_sb, in_=sinb_d)
            nc.sync.dma_start(out=bneg_sb, in_=bneg_d)
            nc.sync.dma_start(out=iden_sb, in_=iden_d)
            nc.sync.dma_start(out=ones_sb, in_=ones_d)

            woc_sb = wpool.tile([128, G, H], BF, tag="woc")
            nc.sync.dma_start(out=woc_sb, in_=woc.rearrange("(c p) o -> p c o", p=128))

            k_chunks = []
            v_chunks = []
            for sc in range(NSC):
                ssl = slice(sc * SC, (sc + 1) * SC)
                if sc == 0:
                    x_sb = x0_sb
                else:
                    x_sb = xpool.tile([128, NHT, SC], BF, tag="x")
                    nc.sync.dma_start(
                        out=x_sb, in_=xT[:, ssl].rearrange("(t p) s -> p t s", p=128)
                    )

                q_sb = qpool.tile([HD, G, SC], BF, tag="q")
                k_c = kvpool.tile([HD, G, SC], BF, tag=f"k{sc}")
                v_c = kvpool.tile([128, 4, GC], BF, tag=f"v{sc}")
                k_chunks.append(k_c)
                v_chunks.append(v_c)

                # ---- Q/K projections + RoPE (all-q first: chunk 0's q work
                # can start as soon as wq + x0 land, before wk arrives) ----
                for w_sb, dst_of in (
                    (wq_sb, lambda h: q_sb[:, h, :]),
                    (wk_sb, lambda h: k_c[:, h, :]),
                ):
                    for h in range(G):
                        dst = dst_of(h)
                        pqk = ps.tile([128, SC], F32, tag="proj", bufs=2)
                        for ht in range(NHT):
                            nc.tensor.matmul(
                                pqk,
                                w_sb[:, ht, h * 128 : (h + 1) * 128],
                                x_sb[:, ht, :],
                                start=(ht == 0),
                                stop=(ht == NHT - 1),
                            )
                        # rope TTs read the PSUM directly: SBUF-SBUF TTs may
                        # not cross partition bases (walrus NCC_IBIR297)
                        tmp = rpool.tile([128, SC], BF, tag="tmp", bufs=2)
                        nc.vector.tensor_tensor(
                            out=tmp[0:64, :], in0=pqk[64:128, :],
                            in1=sin_sb[0:64, ssl], op=MULT,
                        )
                        nc.vector.tensor_tensor(
                            out=tmp[64:128, :], in0=pqk[0:64, :],
                            in1=sin_sb[64:128, ssl], op=MULT,
                        )
                        cp = rpool.tile([128, SC], BF, tag="cp", bufs=2)
                        nc.vector.tensor_tensor(
                            out=cp, in0=pqk, in1=cos_sb[:, ssl], op=MULT
                        )
                        nc.vector.tensor_tensor(out=dst, in0=cp, in1=tmp, op=ADD)

                # ---- V projection ----
                for st2 in range(4):
                    pv = ps.tile([128, SC], F32, tag="proj", bufs=2)
                    for ht in range(NHT):
                        nc.tensor.matmul(
                            pv,
                            x_sb[:, ht, st2 * 128 : (st2 + 1) * 128],
                            wv_sb[:, ht, :],
                            start=(ht == 0),
                            stop=(ht == NHT - 1),
                        )
                    nc.scalar.copy(v_c[:, st2, :], pv)

                # ---- causal attention for q chunk sc ----
                nk = 4 * sc + 4
                outh = opool.tile([HD, G, SC], BF, tag="outh")
                for h in range(G):
                    po = ps.tile([128, SC], F32, tag="po", bufs=2)
                    pbs = ps.tile([128, SC], F32, tag="pbs", bufs=2)
                    for ki in range(nk):
                        kc, kb = divmod(ki, 4)
                        m = ki - 4 * sc
                        qlo = 128 * m if m >= 0 else 0
                        qs = slice(qlo, SC)
                        psc = ps.tile([128, SC], F32, tag="att", bufs=2)
                        nc.tensor.matmul(
                            psc[:, qs],
                            k_chunks[kc][:, h, kb * 128 : (kb + 1) * 128],
                            q_sb[:, h, qs],
                            start=True,
                            stop=(m < 0),
                        )
                        if m >= 0:
                            nc.tensor.matmul(
                                psc[:, qs],
                                iden_sb,
                                bneg_sb[:, m, qs],
                                start=False,
                                stop=True,
                            )
                        pr = prpool.tile([128, SC], BF, tag="pr", bufs=3)
                        nc.scalar.activation(
                            pr[:, qs], psc[:, qs], EXP, scale=INV_SQRT_HD
                        )
                        nc.tensor.matmul(
                            po[:, qs],
                            v_chunks[kc][:, kb, h * 128 : (h + 1) * 128],
                            pr[:, qs],
                            start=(ki == 0),
                            stop=(ki == nk - 1),
                        )
                        nc.tensor.matmul(
                            pbs[:, qs],
                            ones_sb,
                            pr[:, qs],
                            start=(ki == 0),
                            stop=(ki == nk - 1),
                        )
                    # 1/x as exp(-ln(x)) on ACT: the custom-DVE fast
                    # reciprocal doesn't lower in this walrus build, and the
                    # exact DVE reciprocal costs 3.3us per tile.
                    lnb = rpool.tile([128, SC], F32, tag="lnb", bufs=2)
                    nc.scalar.activation(lnb, pbs, LN)
                    bc = rpool.tile([128, SC], F32, tag="bc", bufs=2)
                    nc.scalar.activation(bc, lnb, EXP, scale=-1.0)
                    nc.vector.tensor_tensor(
                        out=outh[:, h, :], in0=po, in1=bc, op=MULT
                    )

                # ---- output projection for chunk sc ----
                for ot in range(NHT):
                    py = ps.tile([128, SC], F32, tag="att", bufs=2)
                    for h in range(G):
                        nc.tensor.matmul(
                            py,
                            woc_sb[:, h, ot * 128 : (ot + 1) * 128],
                            outh[:, h, :],
                            start=(h == 0),
                            stop=(h == G - 1),
                        )
                    ysf = ypool.tile([128, SC], BF, tag="ysf", bufs=3)
                    nc.vector.tensor_copy(ysf, py)
                    nc.sync.dma_start(
                        out=yT[ot * 128 : (ot + 1) * 128, ssl], in_=ysf
                    )

    _split_multi_waits(nc)
    return nc


def _host_tables():
    import ml_dtypes

    BFN = ml_dtypes.bfloat16
    inv_freq = 1.0 / (THETA ** (np.arange(0, HD, 2, dtype=np.float32) / HD))
    t = np.arange(S, dtype=np.float32)
    freqs = np.einsum("i,j->ij", t, inv_freq)  # [S, 64]
    cos_h = np.cos(freqs).astype(np.float32)  # [S, 64]
    sin_h = np.sin(freqs).astype(np.float32)
    cosb = np.empty((HD, S), np.float32)
    cosb[0:64] = cos_h.T
    cosb[64:128] = cos_h.T
    sinb = np.empty((HD, S), np.float32)
    sinb[0:64] = -sin_h.T
    sinb[64:128] = sin_h.T
    p = np.arange(128)[:, None]
    q = np.arange(SC)[None, :]
    bneg = np.empty((128, 4, SC), np.float32)
    for m in range(4):
        bneg[:, m, :] = (q < 128 * m + p).astype(np.float32)
    iden = np.eye(128, dtype=np.float32) * MASKVAL
    ones = np.ones((128, 128), np.float32)
    return {
        "cosb": cosb.astype(BFN),
        "sinb": sinb.astype(BFN),
        "bneg": bneg.astype(BFN),
        "iden": iden.astype(BFN),
        "ones": ones.astype(BFN),
    }


def _in_maps(hidden_states, Wq, Wk, Wv, Wo):
    import ml_dtypes

    BFN = ml_dtypes.bfloat16
    tables = _host_tables()
    maps = []
    for c in range(8):
        b, g = divmod(c, 4)
        rows = slice(g * GC, (g + 1) * GC)
        maps.append(
            {
                "xT": np.ascontiguousarray(hidden_states[b].T).astype(BFN),
                "wqT": np.ascontiguousarray(Wq[rows, :].T).astype(BFN),
                "wkT": np.ascontiguousarray(Wk[rows, :].T).astype(BFN),
                "wvT": np.ascontiguousarray(Wv[rows, :].T).astype(BFN),
                "woc": np.ascontiguousarray(Wo[:, rows].T).astype(BFN),
                **tables,
            }
        )
    return maps


def kernel(hidden_states, Wq, Wk, Wv, Wo):
    from concourse import bass_utils

    hidden_states = np.asarray(hidden_states, dtype=np.float32)
    Wq = np.asarray(Wq, dtype=np.float32)
    Wk = np.asarray(Wk, dtype=np.float32)
    Wv = np.asarray(Wv, dtype=np.float32)
    Wo = np.asarray(Wo, dtype=np.float32)

    if "nc" not in _prog_cache:
        _prog_cache["nc"] = _build_nc()
    nc = _prog_cache["nc"]

    in_maps = _in_maps(hidden_states, Wq, Wk, Wv, Wo)
    res = bass_utils.run_bass_kernel_spmd(
        nc, in_maps, core_ids=list(range(8)), trace=TRACE
    )
    global LAST_RESULTS
    LAST_RESULTS = res

    out = np.zeros((B, S, H), np.float32)
    for c in range(8):
        b = c // 4
        out[b] += res.results[c]["yT"].T.astype(np.float32)
    return out


# revision 10
# speedup vs baseline: 1.4513x; 1.0786x over previous
"""DharmaAttention TRN2 kernel — fused single-pass bf16 pipeline.

Full-input contract: kernel(**inputs) takes the unsharded inputs and returns
the full [2, 2048, 2048] fp32 output.

Sharding (8 cores): 2-way data-parallel over batch x 4-way tensor-parallel
over head groups (4 heads of head_dim 128 per core). Wq/Wk/Wv split
column-wise per head group, Wo row-wise; host sums the 4 partial output
projections per batch element.

v2 design (vs the phase-split fp32r baseline):
  - everything bf16 on the wire and in SBUF (halves DMA + SBUF, enables FWL
    weight loads and 2x DVE modes); PSUM accumulation stays fp32.
  - ONE fused pass per 512-token seq chunk: Q/K proj + RoPE -> V proj ->
    causal attention for that q chunk (k/v of chunks 0..sc stay SBUF
    resident, no DRAM round trip) -> output projection -> DMA out.
  - causal mask applied by an extra accumulate-matmul (-340*I @ B_m) into
    the score PSUM group instead of a DVE multiply; exp then yields ~0.
  - diagonal score blocks only compute the live q range (512-128m cols).
  - softmax denominator: ones-matmul accumulated in PSUM (as baseline), but
    1/x via reciprocal_approx_fast (~5x faster than exact reciprocal).

Per-core DRAM layouts (all bf16):
  xT   [2048, 2048]  hidden_states[b].T          (contraction on partitions)
  wqT  [2048, 512]   Wq[rows of group].T         (same wkT, wvT)
  woc  [512, 2048]   Wo[:, cols of group].T
  cosb [128, 2048]   rope cos table [d, s]
  sinb [128, 2048]   rows 0:64 = -sin, rows 64:128 = +sin
  bneg [128, 4, 512] causal 0/1 tables per diagonal offset m
  iden [128, 128]    -340 * I   (mask add via PE)
  ones [128, 128]    all-ones   (softmax denominator via PE)
Output:
  yT   [2048, 2048]  partial (Wo row-shard) output, [o, s], bf16
"""

import math
import sys

sys.path.insert(0, "/opt/trn_rl_repo")

import numpy as np

B = 2
S = 2048
H = 2048
NH = 16
HD = 128
THETA = 10000.0
G = 4  # heads per core
GC = G * HD  # 512 channels per core
NHT = H // 128  # 16 contraction tiles
SC = 512  # seq chunk
NSC = S // SC  # 4
INV_SQRT_HD = 1.0 / math.sqrt(HD)
MASKVAL = -340.0  # * INV_SQRT_HD ~= -30 after the exp scale

_prog_cache = {}

# test-harness hooks (the grading path leaves these at defaults)
TRACE = False
LAST_RESULTS = None


def _split_multi_waits(nc):
    """The walrus build here accepts at most ONE sync wait per instruction
    ('Too many sync wait commands'). Hoist extra on_wait entries into no-op
    instructions inserted just before, on the same engine."""
    import concourse.mybir as mybir

    for f in nc.m.functions:
        for b in f.blocks:
            out = []
            changed = False
            for inst in b.instructions:
                si = getattr(inst, "sync_info", None)
                waits = list(si.on_wait) if si is not None and si.on_wait else []
                if len(waits) > 1:
                    for k, w in enumerate(waits[:-1]):
                        nop = mybir.InstNoOp(
                            name=f"{inst.name}-w{k}",
                            sync_info=mybir.SyncInfo(on_wait=[w], on_update=[]),
                        )
                        nop.engine = inst.engine
                        out.append(nop)
                    inst.sync_info = mybir.SyncInfo(
                        on_wait=[waits[-1]], on_update=list(si.on_update or [])
                    )
                    changed = True
                out.append(inst)
            if changed:
                b.instructions = out


def _build_nc():
    import concourse.bass as bass
    import concourse.mybir as mybir
    import concourse.tile as tile

    F32 = mybir.dt.float32
    BF = mybir.dt.bfloat16
    F16 = mybir.dt.float16
    MULT = mybir.AluOpType.mult
    ADD = mybir.AluOpType.add
    EXP = mybir.ActivationFunctionType.Exp
    LN = mybir.ActivationFunctionType.Ln

    nc = bass.Bass("TRN2", target_bir_lowering=False, debug=False)

    xT = nc.dram_tensor("xT", [H, S], BF, kind="ExternalInput").ap()
    wqT = nc.dram_tensor("wqT", [H, GC], BF, kind="ExternalInput").ap()
    wkT = nc.dram_tensor("wkT", [H, GC], BF, kind="ExternalInput").ap()
    wvT = nc.dram_tensor("wvT", [H, GC], BF, kind="ExternalInput").ap()
    woc = nc.dram_tensor("woc", [GC, H], BF, kind="ExternalInput").ap()
    cosb_d = nc.dram_tensor("cosb", [HD, S], BF, kind="ExternalInput").ap()
    sinb_d = nc.dram_tensor("sinb", [HD, S], BF, kind="ExternalInput").ap()
    bneg_d = nc.dram_tensor("bneg", [HD, 4, SC], BF, kind="ExternalInput").ap()
    iden_d = nc.dram_tensor("iden", [128, 128], BF, kind="ExternalInput").ap()
    ones_d = nc.dram_tensor("ones", [128, 128], F16, kind="ExternalInput").ap()
    yT = nc.dram_tensor("yT", [H, S], BF, kind="ExternalOutput").ap()

    with tile.TileContext(nc) as tc:
        with (
            tc.tile_pool(name="wpool", bufs=1) as wpool,
            tc.tile_pool(name="consts", bufs=1) as consts,
            tc.tile_pool(name="kvpool", bufs=1) as kvpool,
            tc.tile_pool(name="xpool", bufs=2) as xpool,
            tc.tile_pool(name="qpool", bufs=2) as qpool,
            tc.tile_pool(name="rpool", bufs=1) as rpool,
            tc.tile_pool(name="prpool", bufs=1) as prpool,
            tc.tile_pool(name="opool", bufs=2) as opool,
            tc.tile_pool(name="ypool", bufs=1) as ypool,
            tc.tile_pool(name="ps", bufs=1, space="PSUM") as ps,
        ):
            # DMA issue order is the chunk-0 critical path: the Q projection
            # of chunk 0 needs only wq + x(0), so those go first; everything
            # else streams in behind them while the PE is already busy.
            wq_sb = wpool.tile([128, NHT, GC], BF, tag="wq")
            nc.sync.dma_start(out=wq_sb, in_=wqT.rearrange("(t p) o -> p t o", p=128))
            x0_sb = xpool.tile([128, NHT, SC], BF, tag="x")
            nc.sync.dma_start(
                out=x0_sb, in_=xT[:, 0:SC].rearrange("(t p) s -> p t s", p=128)
            )
            wk_sb = wpool.tile([128, NHT, GC], BF, tag="wk")
            nc.sync.dma_start(out=wk_sb, in_=wkT.rearrange("(t p) o -> p t o", p=128))
            wv_sb = wpool.tile([128, NHT, GC], BF, tag="wv")
            nc.sync.dma_start(out=wv_sb, in_=wvT.rearrange("(t p) o -> p t o", p=128))

            cos_sb = consts.tile([HD, S], BF, tag="cos")
            sin_sb = consts.tile([HD, S], BF, tag="sin")
            bneg_sb = consts.tile([HD, 4, SC], BF, tag="bneg")
            iden_sb = consts.tile([128, 128], BF, tag="iden")
            ones_sb = consts.tile([128, 128], F16, tag="ones")
            nc.sync.dma_start(out=cos_sb, in_=cosb_d)
            nc.sync.dma_start(out=sin_sb, in_=sinb_d)
            nc.sync.dma_start(out=bneg_sb, in_=bneg_d)
            nc.sync.dma_start(out=iden_sb, in_=iden_d)
            nc.sync.dma_start(out=ones_sb, in_=ones_d)

            woc_sb = wpool.tile([128, G, H], BF, tag="woc")
            nc.sync.dma_start(out=woc_sb, in_=woc.rearrange("(c p) o -> p c o", p=128))

            k_chunks = []
            v_chunks = []
            for sc in range(NSC):
                ssl = slice(sc * SC, (sc + 1) * SC)
                if sc == 0:
                    x_sb = x0_sb
                else:
                    x_sb = xpool.tile([128, NHT, SC], BF, tag="x")
                    nc.sync.dma_start(
                        out=x_sb, in_=xT[:, ssl].rearrange("(t p) s -> p t s", p=128)
                    )

                q_sb = qpool.tile([HD, G, SC], BF, tag="q")
                k_c = kvpool.tile([HD, G, SC], BF, tag=f"k{sc}")
                v_c = kvpool.tile([128, 4, GC], F16, tag=f"v{sc}")
                k_chunks.append(k_c)
                v_chunks.append(v_c)

                # ---- Q/K projections + RoPE (all-q first: chunk 0's q work
                # can start as soon as wq + x0 land, before wk arrives) ----
                for w_sb, dst_of in (
                    (wq_sb, lambda h: q_sb[:, h, :]),
                    (wk_sb, lambda h: k_c[:, h, :]),
                ):
                    for h in range(G):
                        dst = dst_of(h)
                        pqk = ps.tile([128, SC], F32, tag="proj", bufs=2)
                        for ht in range(NHT):
                            nc.tensor.matmul(
                                pqk,
                                w_sb[:, ht, h * 128 : (h + 1) * 128],
                                x_sb[:, ht, :],
                                start=(ht == 0),
                                stop=(ht == NHT - 1),
                            )
                        # rope TTs read the PSUM directly: SBUF-SBUF TTs may
                        # not cross partition bases (walrus NCC_IBIR297)
                        tmp = rpool.tile([128, SC], BF, tag="tmp", bufs=2)
                        nc.vector.tensor_tensor(
                            out=tmp[0:64, :], in0=pqk[64:128, :],
                            in1=sin_sb[0:64, ssl], op=MULT,
                        )
                        nc.vector.tensor_tensor(
                            out=tmp[64:128, :], in0=pqk[0:64, :],
                            in1=sin_sb[64:128, ssl], op=MULT,
                        )
                        cp = rpool.tile([128, SC], BF, tag="cp", bufs=2)
                        nc.vector.tensor_tensor(
                            out=cp, in0=pqk, in1=cos_sb[:, ssl], op=MULT
                        )
                        nc.vector.tensor_tensor(out=dst, in0=cp, in1=tmp, op=ADD)

                # ---- V projection ----
                for st2 in range(4):
                    pv = ps.tile([128, SC], F32, tag="proj", bufs=2)
                    for ht in range(NHT):
                        nc.tensor.matmul(
                            pv,
                            x_sb[:, ht, st2 * 128 : (st2 + 1) * 128],
                            wv_sb[:, ht, :],
                            start=(ht == 0),
                            stop=(ht == NHT - 1),
                        )
                    nc.scalar.copy(v_c[:, st2, :], pv)

                # ---- causal attention for q chunk sc ----
                # PE stream is software-pipelined: psc(ki+1) is issued before
                # po(ki), so the exp(ki) latency hides behind the next score
                # block. Per-block denominator matmuls are replaced by fp16
                # DVE accumulation of pr into prsum + ONE ones@prsum matmul
                # per (h, chunk); each head's epilogue (pbs/ln/exp/mult) is
                # emitted inside the next head's first blocks so no engine
                # waits on it in line.
                nk = 4 * sc + 4
                outh = opool.tile([HD, G, SC], BF, tag="outh")
                epilogue = None
                for h in range(G):
                    po = ps.tile([128, SC], F32, tag="po", bufs=2)
                    prsum = rpool.tile([128, SC], F16, tag="prsum", bufs=2)
                    prs = []
                    for ki in range(nk + 1):
                        if ki < nk:
                            kc, kb = divmod(ki, 4)
                            m = ki - 4 * sc
                            qlo = 128 * m if m >= 0 else 0
                            qs = slice(qlo, SC)
                            psc = ps.tile([128, SC], F32, tag="att", bufs=4)
                            nc.tensor.matmul(
                                psc[:, qs],
                                k_chunks[kc][:, h, kb * 128 : (kb + 1) * 128],
                                q_sb[:, h, qs],
                                start=True,
                                stop=(m < 0),
                            )
                            if m >= 0:
                                nc.tensor.matmul(
                                    psc[:, qs],
                                    iden_sb,
                                    bneg_sb[:, m, qs],
                                    start=False,
                                    stop=True,
                                )
                            pr = prpool.tile([128, SC], F16, tag="pr", bufs=4)
                            prs.append((pr, qs, kc, kb))
                            nc.scalar.activation(
                                pr[:, qs], psc[:, qs], EXP, scale=INV_SQRT_HD
                            )
                            if ki == 0:
                                nc.vector.tensor_copy(prsum, pr)
                            else:
                                nc.vector.tensor_tensor(
                                    out=prsum[:, qs], in0=prsum[:, qs],
                                    in1=pr[:, qs], op=ADD,
                                )
                        if ki == 1 and epilogue is not None:
                            epilogue()
                            epilogue = None
                        if ki >= 1:
                            prv, pqs, pkc, pkb = prs[ki - 1]
                            nc.tensor.matmul(
                                po[:, pqs],
                                v_chunks[pkc][:, pkb, h * 128 : (h + 1) * 128],
                                prv[:, pqs],
                                start=(ki == 1),
                                stop=(ki == nk),
                            )

                    def _mk_epilogue(h, po, prsum):
                        def _ep():
                            # 1/x as exp(-ln(x)) on ACT: the custom-DVE fast
                            # reciprocal doesn't lower in this walrus build,
                            # and the exact DVE reciprocal costs 3.3us/tile.
                            pbs = ps.tile([128, SC], F32, tag="att", bufs=4)
                            nc.tensor.matmul(
                                pbs, ones_sb, prsum, start=True, stop=True
                            )
                            lnb = rpool.tile([128, SC], F32, tag="lnb", bufs=2)
                            nc.scalar.activation(lnb, pbs, LN)
                            bc = rpool.tile([128, SC], F32, tag="bc", bufs=2)
                            nc.scalar.activation(bc, lnb, EXP, scale=-1.0)
                            nc.vector.tensor_tensor(
                                out=outh[:, h, :], in0=po, in1=bc, op=MULT
                            )
                        return _ep

                    epilogue = _mk_epilogue(h, po, prsum)
                epilogue()

                # ---- output projection for chunk sc ----
                for ot in range(NHT):
                    py = ps.tile([128, SC], F32, tag="att", bufs=4)
                    for h in range(G):
                        nc.tensor.matmul(
                            py,
                            woc_sb[:, h, ot * 128 : (ot + 1) * 128],
                            outh[:, h, :],
                            start=(h == 0),
                            stop=(h == G - 1),
                        )
                    ysf = ypool.tile([128, SC], BF, tag="ysf", bufs=3)
                    nc.vector.tensor_copy(ysf, py)
                    nc.sync.dma_start(
                        out=yT[ot * 128 : (ot + 1) * 128, ssl], in_=ysf
                    )

    _split_multi_waits(nc)
    return nc


def _host_tables():
    import ml_dtypes

    BFN = ml_dtypes.bfloat16
    inv_freq = 1.0 / (THETA ** (np.arange(0, HD, 2, dtype=np.float32) / HD))
    t = np.arange(S, dtype=np.float32)
    freqs = np.einsum("i,j->ij", t, inv_freq)  # [S, 64]
    cos_h = np.cos(freqs).astype(np.float32)  # [S, 64]
    sin_h = np.sin(freqs).astype(np.float32)
    cosb = np.empty((HD, S), np.float32)
    cosb[0:64] = cos_h.T
    cosb[64:128] = cos_h.T
    sinb = np.empty((HD, S), np.float32)
    sinb[0:64] = -sin_h.T
    sinb[64:128] = sin_h.T
    p = np.arange(128)[:, None]
    q = np.arange(SC)[None, :]
    bneg = np.empty((128, 4, SC), np.float32)
    for m in range(4):
        bneg[:, m, :] = (q < 128 * m + p).astype(np.float32)
    iden = np.eye(128, dtype=np.float32) * MASKVAL
    ones = np.ones((128, 128), np.float32)
    return {
        "cosb": cosb.astype(BFN),
        "sinb": sinb.astype(BFN),
        "bneg": bneg.astype(BFN),
        "iden": iden.astype(BFN),
        "ones": ones.astype(np.float16),
    }


def _in_maps(hidden_states, Wq, Wk, Wv, Wo):
    import ml_dtypes

    BFN = ml_dtypes.bfloat16
    tables = _host_tables()
    maps = []
    for c in range(8):
        b, g = divmod(c, 4)
        rows = slice(g * GC, (g + 1) * GC)
        maps.append(
            {
                "xT": np.ascontiguousarray(hidden_states[b].T).astype(BFN),
                "wqT": np.ascontiguousarray(Wq[rows, :].T).astype(BFN),
                "wkT": np.ascontiguousarray(Wk[rows, :].T).astype(BFN),
                "wvT": np.ascontiguousarray(Wv[rows, :].T).astype(BFN),
                "woc": np.ascontiguousarray(Wo[:, rows].T).astype(BFN),
                **tables,
            }
        )
    return maps


def kernel(hidden_states, Wq, Wk, Wv, Wo):
    from concourse import bass_utils

    hidden_states = np.asarray(hidden_states, dtype=np.float32)
    Wq = np.asarray(Wq, dtype=np.float32)
    Wk = np.asarray(Wk, dtype=np.float32)
    Wv = np.asarray(Wv, dtype=np.float32)
    Wo = np.asarray(Wo, dtype=np.float32)

    if "nc" not in _prog_cache:
        _prog_cache["nc"] = _build_nc()
    nc = _prog_cache["nc"]

    in_maps = _in_maps(hidden_states, Wq, Wk, Wv, Wo)
    res = bass_utils.run_bass_kernel_spmd(
        nc, in_maps, core_ids=list(range(8)), trace=TRACE
    )
    global LAST_RESULTS
    LAST_RESULTS = res

    out = np.zeros((B, S, H), np.float32)
    for c in range(8):
        b = c // 4
        out[b] += res.results[c]["yT"].T.astype(np.float32)
    return out
